# revision 1
# baseline (speedup 1.0000x reference)
"""TRN2 Bass kernel for nn_BalancedHamiltonLayer.

Math: out[n,k,j] = sum_{r,s,i} x[n,s,i] * factors_B[r,j,i] * H(A)[r,k,s] + bias
collapses to a single dense matmul  out = x2d @ W + bias  with
W[(s,i),(k,j)] = sum_r H[r,k,s] * B[r,j,i]  (a 1024x1024 matrix folded on host
in float64).

Sharding: data-parallel over the 8192 token rows across 8 NeuronCores
(1024 rows each); W replicated.  The matmul runs in fp16 on the PE
(full-rate, FWL weight loads, fp32 PSUM accumulation); the output is
stored as fp16 (adds ~5e-4 relative quantization, tolerance is 2e-2)
to halve store traffic, and upcast + bias-added on the host.

Schedule (per core, tuned from NTFF traces):
- every DMA is 2KB-or-1KB/partition contiguous (host packs layouts)
- loads stream on the sync HWDGE queue in just-in-time order; the first
  x half-tile and the two w0 halves go first (w0 on the scalar queue in
  parallel) so the first real matmul can start ~2.5us after body start
- 9 big + 2 short warm-up matmuls on a zeroed tile bridge the DMA
  latency and get the PE HAM clock to 2.4GHz as real data lands; they
  accumulate +0 into the first real PSUM group
- phase 1 k-interleaves m0,m1 then joins m2 (6 MMs per 256KB W chunk =
  1.5x slack vs the ~0.9us wire cadence, so receipt-latency jitter
  doesn't stall the PE); m3..m7 run k-contiguous with n-halves
  staggered so copies/stores drain continuously on the scalar queue
- the final half-tile is stored in two 64KB quarters to cut the tail
"""

import numpy as np
import concourse.bacc as bacc
import concourse.mybir as mybir
import concourse.tile as tile
from concourse.bass_utils import run_bass_kernel_spmd

B, T, D = 4, 2048, 1024
RANK, FACTOR, SUB = 8, 64, 4
S = 4 * SUB  # 16
NCORES = 8
NTOK = B * T // NCORES  # 1024 token rows per core
P = 128
KT = D // P     # 8 contraction chunks
MT = NTOK // P  # 8 token tiles per core
NH = 512        # f_out half (one PSUM bank)

_cached_nc = None


def build_module():
    global _cached_nc
    if _cached_nc is not None:
        return _cached_nc
    nc = bacc.Bacc("TRN2", target_bir_lowering=False, debug=False)
    xH = nc.dram_tensor("xH", [MT, P, KT, P], mybir.dt.float16, kind="ExternalInput").ap()
    wH = nc.dram_tensor("wH", [KT, P, D], mybir.dt.float16, kind="ExternalInput").ap()
    out = nc.dram_tensor("out", [NTOK, D], mybir.dt.float16, kind="ExternalOutput").ap()

    with tile.TileContext(nc) as tc:
        with (
            tc.tile_pool(name="wp", bufs=1) as wp,
            tc.tile_pool(name="xp", bufs=1) as xp,
            tc.tile_pool(name="op", bufs=1) as op,
            tc.tile_pool(name="ps", bufs=8, space="PSUM") as ps,
        ):
            g = xp.tile([P, NH], mybir.dt.float16, tag="warm", name="g")
            nc.vector.memset(g[:], 0.0)

            KH = KT // 2
            x0a = xp.tile([P, KH, P], mybir.dt.float16, tag="x0a", name="x0a")
            x0b = xp.tile([P, KH, P], mybir.dt.float16, tag="x0b", name="x0b")
            w0a = wp.tile([P, NH], mybir.dt.float16, tag="w0a", name="w0a")
            w0b = wp.tile([P, NH], mybir.dt.float16, tag="w0b", name="w0b")
            xt = {}
            wt = {}
            for m in range(1, MT):
                xt[m] = xp.tile([P, KT, P], mybir.dt.float16, tag=f"x{m}", name=f"xt{m}")
            for k in range(1, KT):
                wt[k] = wp.tile([P, D], mybir.dt.float16, tag=f"w{k}", name=f"wt{k}")

            # w0 halves go on the scalar queue: the two queues overlap
            # issue+completion latency so the first ~0.7MB lands faster
            # than a single queue can deliver it.  Everything else streams
            # on sync in strict just-in-time order.
            nc.sync.dma_start(x0a[:], xH[0, :, :KH])
            nc.scalar.dma_start(w0a[:], wH[0, :, :NH])
            nc.scalar.dma_start(w0b[:], wH[0, :, NH:])
            for da, sa in [
                (xt[1], xH[1]),
                (wt[1], wH[1]),
                (xt[2], xH[2]),
                (wt[2], wH[2]),
                (x0b, xH[0, :, KH:]),
                (wt[3], wH[3]),
                (wt[4], wH[4]),
                (xt[3], xH[3]),
                (wt[5], wH[5]),
                (wt[6], wH[6]),
                (wt[7], wH[7]),
                (xt[4], xH[4]),
                (xt[5], xH[5]),
                (xt[6], xH[6]),
                (xt[7], xH[7]),
            ]:
                nc.sync.dma_start(da[:], sa)

            def xs(m, k):
                if m == 0:
                    return x0a[:, k, :] if k < KH else x0b[:, k - KH, :]
                return xt[m][:, k, :]

            def ws(k, n):
                if k == 0:
                    return (w0a if n == 0 else w0b)[:]
                return wt[k][:, n * NH:(n + 1) * NH]

            def wsq(k, n, q, NQ):
                if k == 0:
                    return (w0a, w0b)[n][:, q * NQ:(q + 1) * NQ]
                c0 = n * NH + q * NQ
                return wt[k][:, c0:c0 + NQ]

            ot = {}

            def emit_piece(m, c0, c1, pt, eng):
                # n0 halves store via the scalar queue, n1 via sync (idle
                # after the loads) so store issues never queue up behind
                # each other at the kernel tail
                if m not in ot:
                    ot[m] = op.tile([P, D], mybir.dt.float16, tag=f"o{m}", name=f"o{m}")
                o = ot[m]
                nc.vector.tensor_copy(o[:, c0:c1], pt[:])
                eng.dma_start(out[m * P:(m + 1) * P, c0:c1], o[:, c0:c1])

            def emit_half(m, n, pt):
                emit_piece(m, n * NH, (n + 1) * NH, pt, nc.scalar if n == 0 else nc.sync)

            with nc.named_scope("mm"):
                # Warm-up must cover from body start (~7.6us into the
                # window) until the first loads are consumable (~11.4us:
                # issue + wire + ~2.5us completion/sem latency).  A PE gap
                # here resets the HAM activity window and the whole phase-1
                # runs at 1.2GHz (measured +2.7us), so overshoot slightly
                # and finish with short 128-col dummies for granularity.
                NWARM_BIG, NWARM_SMALL = 9, 4
                pts = {
                    m: {
                        n: ps.tile([P, NH], mybir.dt.float32, tag="ps", name=f"pt{m}_{n}")
                        for n in range(2)
                    }
                    for m in range(3)
                }
                for i in range(NWARM_BIG):
                    nc.tensor.matmul(
                        pts[0][0][:], g[:, :P], g[:], start=(i == 0), stop=False
                    )
                for i in range(NWARM_SMALL):
                    nc.tensor.matmul(
                        pts[0][0][:, :P], g[:, :P], g[:, :P], start=False, stop=False
                    )

                def mm(m, k, n):
                    nc.tensor.matmul(
                        pts[m][n][:],
                        xs(m, k),
                        ws(k, n),
                        start=(k == 0 and not (m == 0 and n == 0)),
                        stop=(k == KT - 1),
                    )

                # phase 1: m0,m1 lead; m2 catches up after k1; then 6 MMs
                # per W chunk (1.5x slack vs the wire cadence)
                for k in (0, 1):
                    for m in (0, 1):
                        for n in range(2):
                            mm(m, k, n)
                for k in (0, 1):
                    for n in range(2):
                        mm(2, k, n)
                for k in range(2, KT):
                    for m in (0, 1, 2):
                        for n in range(2):
                            mm(m, k, n)
                for m in (0, 1, 2):
                    for n in range(2):
                        emit_half(m, n, pts[m][n])

                # phase 2: k-contiguous, n-halves staggered so copies and
                # stores drain while the other half's matmuls run
                for m in range(3, MT):
                    last = m == MT - 1
                    pt = {
                        n: ps.tile([P, NH], mybir.dt.float32, tag="ps", name=f"pt{m}_{n}")
                        for n in range(2)
                    }
                    for k in range(KT):
                        nc.tensor.matmul(
                            pt[0][:], xs(m, k), ws(k, 0),
                            start=(k == 0), stop=(k == KT - 1),
                        )
                    emit_half(m, 0, pt[0])
                    if not last:
                        for k in range(KT):
                            nc.tensor.matmul(
                                pt[1][:], xs(m, k), ws(k, 1),
                                start=(k == 0), stop=(k == KT - 1),
                            )
                        emit_half(m, 1, pt[1])
                    else:
                        # final half in two quarter accumulation groups so
                        # the very last copy+store is small and its issue
                        # goes to the empty sync queue
                        NQ = NH // 2
                        ptq = [
                            ps.tile([P, NQ], mybir.dt.float32, tag="ps", name=f"ptq{q}")
                            for q in range(2)
                        ]
                        for q in range(2):
                            for k in range(KT):
                                nc.tensor.matmul(
                                    ptq[q][:],
                                    xs(m, k),
                                    wsq(k, 1, q, NQ),
                                    start=(k == 0), stop=(k == KT - 1),
                                )
                            emit_piece(m, NH + q * NQ, NH + (q + 1) * NQ, ptq[q], nc.sync)
    nc.compile()
    _cached_nc = nc
    return nc


def _construct_hamilton(A):
    # A: [rank, 4, sub, sub] -> [rank, 4*sub, 4*sub]
    r, i, j, k = A[:, 0], A[:, 1], A[:, 2], A[:, 3]
    return np.concatenate(
        [
            np.concatenate([r, -i, -j, -k], axis=2),
            np.concatenate([i, r, -k, j], axis=2),
            np.concatenate([j, k, r, -i], axis=2),
            np.concatenate([k, -j, i, r], axis=2),
        ],
        axis=1,
    )


def build_in_maps(x, A, factors_B):
    H = _construct_hamilton(np.asarray(A, dtype=np.float64))  # [r, k, s]
    Bf = np.asarray(factors_B, dtype=np.float64)  # [r, j, i]
    # W[(s,i),(k,j)] = sum_r H[r,k,s] * B[r,j,i]
    W = np.einsum("rks,rji->sikj", H, Bf).reshape(D, D).astype(np.float16)
    wH = np.ascontiguousarray(W.reshape(KT, P, D))

    x2 = np.asarray(x, dtype=np.float16).reshape(NCORES, NTOK, D)
    in_maps = []
    for c in range(NCORES):
        # [NTOK, D] -> [m, t, k, p] -> [m, p, k, t] so each per-m DMA is
        # 2KB/partition contiguous
        xs_ = np.ascontiguousarray(
            x2[c].reshape(MT, P, KT, P).transpose(0, 3, 2, 1)
        )
        in_maps.append({"xH": xs_, "wH": wH})
    return in_maps


def kernel(x, A, factors_B, bias):
    nc = build_module()
    in_maps = build_in_maps(x, A, factors_B)
    br = run_bass_kernel_spmd(nc, in_maps, core_ids=list(range(NCORES)))
    out = np.concatenate([r["out"] for r in br.results], axis=0)
    out = out.astype(np.float32) + np.asarray(bias, dtype=np.float32)[None, :]
    return out.reshape(B, T, D)

